# revision 1
# baseline (speedup 1.0000x reference)
"""Trainium2 Bass kernel for CapsNet DigitCaps dynamic routing (nn_DigitCaps).

Reference computation:
    u_hat[b,r,j,o] = W[r,j,o,:] @ x[b,r,:]        B,R,J,O,I = 512,1152,10,16,8
    b_ij = 0; 3 routing iterations:
        c = softmax(b_ij, axis=0)                  # over routes r, per j
        s[b,j,o] = sum_r c[r,j] * u_hat[b,r,j,o]
        v = squash(s) = s*|s|/(1+s^2)              # elementwise
        b_ij += mean_b sum_o u_hat[b,r,j,o]*v[b,j,o]
    return v[..., None]

Kernel strategy (data-parallel over batch, 8 cores, 64 rows each; u_hat is
never materialized). All matmul/elementwise operands fp16 (PE: 1 cyc per
output row at any N; DVE 2x/4x fast modes need 2-byte packed SBUF operands),
fp32 PSUM accumulation. rel err ~5e-3 vs the fp32 reference.

r-major layout: partition p = r % 128, free blocks rb = r // 128 (9 blocks),
so b_ij, e=exp(b), and b_upd all live as [128, (rb j)] = [128, 90] with no
cross-partition shuffles:
    s    = X @ (e-scaled W2); e expanded over o once per iter (eo) so the
           wc broadcast-mult is innermost-packed; softmax 1/Z deferred to
           squash (it commutes through the matmul).
    M_k  = X_k^T @ V per (rb,i) chunk; 3 of 4 PSUM blocks are ACT-copied to
           fp16 SBUF so the W2R (*) M products run at the DVE fast rate.
    b_upd[:, rb*10:..] = tree-add over i + small XY-reduce over (i,o).
    b_upd is AllGather-summed across cores (~15us NRT collective); softmax
    runs redundantly on every core.

Each rep is split into 7 stages A..G (s0 / M0 / AR0 / s1 / M1 / AR1 / s2)
and emitted as a modulo software pipeline across reps (rep r's stage s in
slot r+s): per-engine instruction queues execute in emission order, so the
interleave is what lets other reps' compute fill each rep's two collective
windows.
"""
import os
import numpy as np
from contextlib import ExitStack

import concourse.bacc as bacc
import concourse.bass as bass
import concourse.tile as tile
from concourse import mybir
from concourse.bass_utils import run_bass_kernel_spmd

F32 = mybir.dt.float32
F16 = mybir.dt.float16

B, R, J, O, I = 512, 1152, 10, 16, 8
N_CORES = 8
BL = B // N_CORES          # 64 batch rows per core
RI = R * I                 # 9216
NJO = J * O                # 160
NRB = 9                    # r-blocks of 128
KT = RI // 128             # 72 contraction chunks (= NRB * I)
NUM_ITER = 3
MST = 256                  # mps chunk stride (PSUM bank alignment)
NST = 7                    # pipeline stages per rep


def make_stages(nc, tc, tensors, pools, out_d, flags):
    (XT_s, X2_s, W2R_s, ONESB_s, RONESB_s) = tensors
    (sp, vq, wc_pool, p_pool, dram_pool, ps_s, ps_m, ps_z) = pools
    st = {}

    def emit_post_ar(it):
        """exp + eo + Z chain preparing iteration `it` (reads st['b_state'])."""
        e_s = sp.tile([128, 90], F16, tag="e", name="e_s")
        nc.scalar.activation(e_s[:], st['b_state'][:],
                             mybir.ActivationFunctionType.Exp,
                             scale=1.0 / B)
        eo = sp.tile([128, 90 * O], F16, tag="eo", name="eo")
        i0 = e_s[:].rearrange("p (rb j a) -> p rb j a", j=J, a=1)
        eov = eo[:].rearrange("p (rb j o) -> p rb j o", j=J, o=O)
        i0b, _ = bass.broadcast_tensor_aps(i0, eov)
        nc.scalar.activation(eov, i0b, mybir.ActivationFunctionType.Copy)
        zmix = ps_z.tile([BL, 100], F32, tag="z")
        zps = zmix[0:1, 0:90]
        nc.tensor.matmul(zps, ONESB_s[:, 0:1], e_s[:], start=True, stop=True)
        zsum = sp.tile([1, 10], F32, tag="zsum")
        nc.vector.tensor_reduce(
            zsum[:], zps.rearrange("p (rb j) -> p j rb", j=J),
            axis=mybir.AxisListType.X, op=mybir.AluOpType.add)
        zinv = sp.tile([1, 10], F32, tag="zinv")
        nc.vector.reciprocal(zinv[:], zsum[:])
        zinv16 = sp.tile([1, 10], F16, tag="zinv16")
        nc.vector.tensor_copy(zinv16[:], zinv[:])
        zb_ps = zmix[0:BL, 90:100]
        nc.tensor.matmul(zb_ps, RONESB_s[:, 0:BL], zinv16[:],
                         start=True, stop=True)
        zinv_b = sp.tile([BL, 10], F32, tag="zinv_b")
        nc.vector.tensor_copy(zinv_b[:], zb_ps)
        st[f'eo{it}'] = eo
        st[f'zinv_b{it}'] = zinv_b

    def stage_s(it):
        def f():
            scaled = it > 0 and "skip_scale" not in flags
            s_ps = ps_s.tile([BL, NJO], F32, tag="s")
            for rb in range(NRB):
                if scaled:
                    eo = st[f'eo{it}']
                    wc = wc_pool.tile([128, I * NJO], F16, tag="wc")
                    in0 = W2R_s[:, rb * I * NJO:(rb + 1) * I * NJO] \
                        .rearrange("p (i j o) -> p i j o", j=J, o=O)
                    in1 = eo[:, rb * J * O:(rb + 1) * J * O] \
                        .rearrange("p (a j o) -> p a j o", a=1, o=O)
                    i0b, i1b = bass.broadcast_tensor_aps(in0, in1)
                    nc.vector.tensor_tensor(
                        wc[:].rearrange("p (i j o) -> p i j o", j=J, o=O),
                        i0b, i1b, op=mybir.AluOpType.mult)
                for i in range(I):
                    k = rb * I + i
                    rhs = (wc[:, i * NJO:(i + 1) * NJO] if scaled
                           else W2R_s[:, k * NJO:(k + 1) * NJO])
                    nc.tensor.matmul(s_ps[:], XT_s[:, k * BL:(k + 1) * BL],
                                     rhs, start=(k == 0), stop=(k == KT - 1))

            # squash (deferred softmax normalization when scaled), fp16
            # chain after a single PSUM read; kscl folded into s16
            s16 = sp.tile([BL, NJO], F16, tag="s16", name="s16")
            if scaled:
                zinv_b = st[f'zinv_b{it}']
                i0 = s_ps[:].rearrange("p (j o) -> p j o", o=O)
                i1 = zinv_b[:].rearrange("p (j o) -> p j o", o=1)
                i0b, i1b = bass.broadcast_tensor_aps(i0, i1)
                nc.vector.tensor_tensor(
                    s16[:].rearrange("p (j o) -> p j o", o=O), i0b, i1b,
                    op=mybir.AluOpType.mult)
            else:
                nc.scalar.activation(s16[:], s_ps[:],
                                     mybir.ActivationFunctionType.Copy,
                                     scale=1.0 / R)
            src = s16
            with nc.allow_low_precision(reason="fp16 squash; |s|<20"):
                sneg = sp.tile([BL, NJO], F16, tag="sneg")
                nc.vector.tensor_scalar_mul(sneg[:], src[:], -1.0)
                sabs = sp.tile([BL, NJO], F16, tag="sabs")
                nc.vector.tensor_tensor(sabs[:], src[:], sneg[:],
                                        op=mybir.AluOpType.max)
                den1 = sp.tile([BL, NJO], F16, tag="den1")
                nc.vector.scalar_tensor_tensor(den1[:], src[:], 1.0,
                                               src[:],
                                               op0=mybir.AluOpType.mult,
                                               op1=mybir.AluOpType.mult)
                nc.vector.tensor_scalar_add(den1[:], den1[:], 1.0)
                rec = sp.tile([BL, NJO], F16, tag="rec")
                nc.vector.reciprocal(rec[:], den1[:])
                num = sp.tile([BL, NJO], F16, tag="num")
                nc.vector.tensor_mul(num[:], src[:], sabs[:])
            if it == NUM_ITER - 1:
                vout = vq.tile([BL, NJO], F32, tag="vout")
                nc.vector.tensor_mul(vout[:], num[:], rec[:])
                nc.sync.dma_start(out_d[:], vout[:])
            else:
                vpad = vq.tile([BL, NJO], F16, tag="vpad")
                nc.vector.tensor_mul(vpad[:], num[:], rec[:])
                st[f'vpad{it}'] = vpad
        return f

    def stage_m(it):
        def f():
            if "skip_m" in flags:
                b_upd = sp.tile([128, 90], F16, tag="b_upd")
                nc.vector.memset(b_upd[:], 0.001)
                st[f'b_upd{it}'] = b_upd
                return
            vpad = st[f'vpad{it}']
            b_upd = sp.tile([128, 90], F16, tag="b_upd")
            for rb in range(NRB):
                prb = p_pool.tile([128, I * NJO], F16, tag="prb")
                for half in range(2):
                    # two 2-chunk PSUM tiles ACT-copied into one 4-chunk fp16
                    # buffer, then a single batched DVE product
                    mcp = p_pool.tile([128, 4 * NJO], F16, tag="mcp")
                    for quarter in range(2):
                        mps = ps_m.tile([128, 2 * MST], F32, tag="m")
                        for q in range(2):
                            k = rb * I + half * 4 + quarter * 2 + q
                            nc.tensor.matmul(mps[:, q * MST:q * MST + NJO],
                                             X2_s[:, k * 128:(k + 1) * 128],
                                             vpad[:], start=True, stop=True)
                        mview = mps[:].rearrange("p (c n) -> p c n",
                                                 n=MST)[:, :, 0:NJO]
                        nc.scalar.activation(
                            mcp[:, quarter * 2 * NJO:(quarter + 1) * 2 * NJO]
                            .rearrange("p (c n) -> p c n", n=NJO),
                            mview, mybir.ActivationFunctionType.Copy)
                    c0 = half * 4
                    nc.vector.tensor_tensor(
                        prb[:, c0 * NJO:(c0 + 4) * NJO],
                        W2R_s[:, (rb * I + c0) * NJO:(rb * I + c0 + 4) * NJO],
                        mcp[:], op=mybir.AluOpType.mult)
                t1 = p_pool.tile([128, 4 * NJO], F16, tag="t1", name="t1")
                nc.vector.tensor_add(t1[:], prb[:, 0:4 * NJO],
                                     prb[:, 4 * NJO:8 * NJO])
                t2 = p_pool.tile([128, 2 * NJO], F16, tag="t2", name="t2")
                nc.vector.tensor_add(t2[:], t1[:, 0:2 * NJO],
                                     t1[:, 2 * NJO:4 * NJO])
                with nc.allow_low_precision(reason="fp16 b_upd; 0.05% rel"):
                    nc.vector.tensor_reduce(
                        b_upd[:, rb * J:(rb + 1) * J],
                        t2[:].rearrange("p (i j o) -> p j i o", j=J, o=O),
                        axis=mybir.AxisListType.XY, op=mybir.AluOpType.add)
            st[f'b_upd{it}'] = b_upd
        return f

    def stage_ar(it):
        def f():
            b_upd = st[f'b_upd{it}']
            cc_in = dram_pool.tile([128, 90], F16, tag="cc_in")
            nc.sync.dma_start(cc_in[:], b_upd[:])
            if "skip_ar" in flags:
                cc_out = dram_pool.tile([128, 90], F16, tag="cc_out")
                nc.sync.dma_start(cc_out[:], cc_in[:])
                gath = sp.tile([128, 8 * 90], F16, tag="gath")
                for kk in range(8):
                    nc.sync.dma_start(gath[:, kk * 90:(kk + 1) * 90],
                                      cc_out[:])
            else:
                cc_out = dram_pool.tile([N_CORES * 128, 90], F16,
                                        tag="cc_outg")
                nc.gpsimd.collective_compute(
                    "AllGather", mybir.AluOpType.bypass,
                    replica_groups=[list(range(N_CORES))],
                    ins=[cc_in.opt()], outs=[cc_out.opt()])
                gath = sp.tile([128, 8 * 90], F16, tag="gath")
                nc.sync.dma_start(
                    gath[:].rearrange("p (k f) -> p k f", f=90),
                    cc_out[:].rearrange("(k p) f -> p k f", p=128))
            if it == 0:
                b_state = sp.tile([128, 90], F16, tag="bstate0")
                with nc.allow_low_precision(reason="fp16 b state"):
                    nc.vector.tensor_reduce(
                        b_state[:],
                        gath[:].rearrange("p (k f) -> p f k", f=90),
                        axis=mybir.AxisListType.X, op=mybir.AluOpType.add)
            else:
                upd_g = sp.tile([128, 90], F16, tag="upd_g")
                with nc.allow_low_precision(reason="fp16 b state"):
                    nc.vector.tensor_reduce(
                        upd_g[:],
                        gath[:].rearrange("p (k f) -> p f k", f=90),
                        axis=mybir.AxisListType.X, op=mybir.AluOpType.add)
                b_state = sp.tile([128, 90], F16, tag="bstate1b")
                nc.vector.tensor_add(b_state[:], st['b_state'][:], upd_g[:])
            st['b_state'] = b_state
            emit_post_ar(it + 1)
        return f

    return [stage_s(0), stage_m(0), stage_ar(0),
            stage_s(1), stage_m(1), stage_ar(1),
            stage_s(2)]


def build_nc(reps=1, flags=()):
    nc = bacc.Bacc("TRN2", target_bir_lowering=False, debug=False,
                   num_devices=N_CORES)
    XT_d = nc.dram_tensor("XT", [128, KT * BL], F16, kind="ExternalInput")
    X2_d = nc.dram_tensor("X2", [BL, RI], F16, kind="ExternalInput")
    W2R_d = nc.dram_tensor("W2R", [128, KT * NJO], F16, kind="ExternalInput")
    ONESB_d = nc.dram_tensor("ONESB", [128, 1], F16, kind="ExternalInput")
    RONESB_d = nc.dram_tensor("RONESB", [1, BL], F16, kind="ExternalInput")
    out_d = nc.dram_tensor("out", [BL, NJO], F32, kind="ExternalOutput")

    with tile.TileContext(nc) as tc:
        with ExitStack() as ctx:
            pers = ctx.enter_context(tc.tile_pool(name="pers", bufs=1))
            sp = ctx.enter_context(tc.tile_pool(name="sp", bufs=4))
            vq = ctx.enter_context(tc.tile_pool(name="vq", bufs=3))
            wc_pool = ctx.enter_context(tc.tile_pool(name="wcp", bufs=3))
            p_pool = ctx.enter_context(tc.tile_pool(name="pp", bufs=3))
            dram_pool = ctx.enter_context(
                tc.tile_pool(name="dram", bufs=3, space="DRAM"))
            ps_s = ctx.enter_context(tc.tile_pool(name="ps_s", bufs=3, space="PSUM"))
            ps_m = ctx.enter_context(tc.tile_pool(name="ps_m", bufs=3, space="PSUM"))
            ps_z = ctx.enter_context(tc.tile_pool(name="ps_z", bufs=2, space="PSUM"))

            XT_s = pers.tile([128, KT * BL], F16)
            X2_s = pers.tile([BL, RI], F16)
            W2R_s = pers.tile([128, KT * NJO], F16)
            ONESB_s = pers.tile([128, 1], F16)
            RONESB_s = pers.tile([1, BL], F16)

            for g in range(3):
                nc.sync.dma_start(
                    XT_s[:, g * 24 * BL:(g + 1) * 24 * BL],
                    XT_d[:, g * 24 * BL:(g + 1) * 24 * BL])
                nc.sync.dma_start(
                    X2_s[:, g * 3072:(g + 1) * 3072],
                    X2_d[:, g * 3072:(g + 1) * 3072])
            for g in range(6):
                nc.sync.dma_start(
                    W2R_s[:, g * 12 * NJO:(g + 1) * 12 * NJO],
                    W2R_d[:, g * 12 * NJO:(g + 1) * 12 * NJO])
            nc.sync.dma_start(ONESB_s[:], ONESB_d[:])
            nc.sync.dma_start(RONESB_s[:], RONESB_d[:])

            tensors = (XT_s, X2_s, W2R_s, ONESB_s, RONESB_s)
            pools = (sp, vq, wc_pool, p_pool, dram_pool, ps_s, ps_m, ps_z)

            # modulo software pipeline: rep r's stage s lands in slot r+s
            stage_lists = [None] * reps
            for slot in range(reps + NST - 1):
                for s in range(NST - 1, -1, -1):
                    r = slot - s
                    if 0 <= r < reps:
                        if stage_lists[r] is None:
                            stage_lists[r] = make_stages(
                                nc, tc, tensors, pools, out_d, flags)
                        stage_lists[r][s]()

    nc.compile()
    return nc


def make_host_inputs(x, W):
    """Build per-core in_maps from the full inputs (r-major fp16 layouts)."""
    x = np.ascontiguousarray(np.asarray(x, dtype=np.float32))
    W = np.asarray(W, dtype=np.float32)
    f16 = np.float16
    # W2R[p, rb, i, j, o] = W[rb*128+p, j, o, i]
    W2R = np.ascontiguousarray(
        W.reshape(NRB, 128, J, O, I).transpose(1, 0, 4, 2, 3)
        .reshape(128, KT * NJO)).astype(f16)
    ONESB = np.ones((128, 1), f16)
    RONESB = np.ones((1, BL), f16)

    in_maps = []
    for c in range(N_CORES):
        xc = x[c * BL:(c + 1) * BL]                      # [64, R, I]
        XT = np.ascontiguousarray(
            xc.transpose(1, 2, 0).reshape(NRB, 128, I, BL)
            .transpose(1, 0, 2, 3).reshape(128, KT * BL)).astype(f16)
        X2 = np.ascontiguousarray(
            xc.reshape(BL, NRB, 128, I).transpose(0, 1, 3, 2)
            .reshape(BL, RI)).astype(f16)
        in_maps.append({
            "XT": XT,
            "X2": X2,
            "W2R": W2R,
            "ONESB": ONESB,
            "RONESB": RONESB,
        })
    return in_maps


def assemble_output(results):
    return np.concatenate(
        [results[c]["out"].reshape(BL, J, O, 1) for c in range(N_CORES)],
        axis=0).astype(np.float32)


_NC_CACHE = {}


def kernel(x, W):
    if "nc" not in _NC_CACHE:
        _NC_CACHE["nc"] = build_nc(reps=1)
    nc = _NC_CACHE["nc"]
    in_maps = make_host_inputs(x, W)
    res = run_bass_kernel_spmd(nc, in_maps, list(range(N_CORES)))
    return assemble_output(res.results)


if __name__ == "__main__":
    import reference
    inputs = reference.setup_inputs()
    expected = np.asarray(reference.reference(**inputs))
    got = kernel(np.asarray(inputs["x"]), np.asarray(inputs["W"]))
    err = np.abs(got - expected).max()
    rel = err / np.abs(expected).max()
    print("abs err:", err, "scale-rel err:", rel)

